# revision 16
# baseline (speedup 1.0000x reference)
"""Trainium2 Bass kernel for CustomCenterQuantizerLinear.

Computes out = x @ f(weight_q).T + bias over 8 NeuronCores, where f is the
piecewise dequantizer:
    y = q / scale
    f = sign(y) * (eps + |y|*(gam-eps))        for |y| <= 1
    f = sign(y) * gam * exp(|y| - 1)           for |y| >  1
    f = 0                                      for y == 0

Sharding: tensor-parallel column split of weight/bias over out_features
(1024 per core), x replicated.

Math (in f' = f/alpha units, alpha=(gam-eps)/sc, K=eps/alpha, G=gam/alpha):
    f'(q) = s*(max(E,G) - (G-K)) + clamp(q, +-(G-K)),  E = G*exp(|q|/sc - 1)
which is exact for integer q (f'(0)=K, an accepted eps-size error matching
the |y|==0 branch within the rel-err budget).

On-chip, per weight tile (s = sign bit of q):
    sE  = s | (max(exp(|q|*inv_s + B0), G) - (G-K))     [1 exp + 3 TS + 1 TT]
    c8  = fp8e4(clamp(q, +-(G-K)))                      [Pool dual TS]
    out = x' @ sE  (bf16 matmul)  +  x8 @ c8 (fp8 DoubleRow matmul, 2x PE)
with the two PSUM streams merged as psA + (alpha/sigma)*psB.

Weights are pre-scaled on host by sigma = 32/(sc*(1-eps/gam)) so the clamp
bound lands exactly on 32 (representable in fp8e4m3) -- this kills the
systematic fp8 rounding bias at the clamp plateau.
"""

import math
import sys

sys.path.insert(0, "/opt/trn_rl_repo")

import numpy as np
from ml_dtypes import bfloat16, float8_e4m3fn

B, S, IN, OUT = 8, 32, 8192, 8192
N_CORES = 8
M = B * S                 # 256 tokens
O_SH = OUT // N_CORES     # 1024 out features per core
KB = 128                  # contraction block (PE partition dim)
NKB = IN // KB            # 64 k-blocks
MB = 128                  # token block (PSUM partition dim)
NMB = M // MB             # 2 token blocks
OC = 512                  # matmul free-dim chunk (one PSUM bank)
NOC = O_SH // OC          # 2 chunks
NH = 4                    # k-blocks per weight tile
NKP = NKB // NH           # 16 weight tiles
W2 = NH * O_SH            # 4096 free elems per weight tile

_CACHE = {}


def _build(inv_s, b0, g, cb, merge_scale):
    """inv_s/b0/g are in sigma-scaled units; cb is the clamp bound (32)."""
    import concourse.bass as bass
    import concourse.bacc as bacc
    import concourse.mybir as mybir
    import concourse.tile as tile

    BF = mybir.dt.bfloat16
    F32 = mybir.dt.float32
    F8 = mybir.dt.float8e4
    U16 = mybir.dt.uint16
    U32 = mybir.dt.uint32
    Alu = mybir.AluOpType
    Act = mybir.ActivationFunctionType
    DR = mybir.MatmulPerfMode.DoubleRow

    nc = bacc.Bacc("TRN2", target_bir_lowering=False, debug=False,
                   num_devices=N_CORES)
    wT_d = nc.dram_tensor("wT", [KB, NKB * O_SH], BF, kind="ExternalInput").ap()
    xT_d = nc.dram_tensor("xT", [KB, NKB * M], BF, kind="ExternalInput").ap()
    x8_d = nc.dram_tensor("x8", [KB, NKB * M], F8, kind="ExternalInput").ap()
    bias_d = nc.dram_tensor("bias", [1, O_SH], BF, kind="ExternalInput").ap()
    out_d = nc.dram_tensor("out", [M, O_SH], F32, kind="ExternalOutput").ap()

    with tile.TileContext(nc) as tc:
        with (
            tc.tile_pool(name="misc", bufs=1) as misc,
            tc.tile_pool(name="wp", bufs=6) as wp,
            tc.tile_pool(name="aqp", bufs=2) as aqp,
            tc.tile_pool(name="ep", bufs=3) as ep,
            tc.tile_pool(name="sp", bufs=4) as sp,
            tc.tile_pool(name="cp", bufs=4) as cp,
            tc.tile_pool(name="psum", bufs=1, space=bass.MemorySpace.PSUM) as pp,
        ):
            xT_sb = misc.tile([KB, NKB * M], BF)
            x8_sb = misc.tile([KB, NKB, M], F8)
            bias_sb = misc.tile([1, O_SH], BF)
            ones_sb = misc.tile([1, MB], BF)
            b0c = misc.tile([128, 1], F32)
            nc.vector.memset(ones_sb[:], 1.0)
            nc.vector.memset(b0c[:], b0)

            psA = [pp.tile([MB, O_SH], F32, name=f"psA{mi}", tag=f"psA{mi}")
                   for mi in range(NMB)]
            psB = [pp.tile([MB, O_SH], F32, name=f"psB{mi}", tag=f"psB{mi}")
                   for mi in range(NMB)]

            LAG = 1          # stages between prep (aq/sgn/exp/c8) and consume
            PF = 3           # weight-DMA prefetch depth beyond prep stage
            wts, aqs, es, sgns, c8s = {}, {}, {}, {}, {}

            def dma_w(t):
                if t < NKP:
                    wts[t] = wp.tile([KB, W2], BF, name="wt")
                    nc.sync.dma_start(wts[t][:],
                                      wT_d[:, t * W2:(t + 1) * W2])

            def do_matmuls(u):
                c8 = c8s.pop(u)
                sgn = sgns.pop(u)
                wts.pop(u)
                # fp8 DoubleRow stream first (inputs ready earliest)
                for i in range(NH // 2):
                    pg = (NH // 2) * u + i
                    for mi in range(NMB):
                        lhs8 = x8_sb[:, 2 * pg:2 * pg + 2,
                                     mi * MB:(mi + 1) * MB]
                        for oc in range(NOC):
                            nc.tensor.matmul(
                                psB[mi][:, oc * OC:(oc + 1) * OC],
                                lhs8,
                                c8[:, 2 * i:2 * i + 2, oc * OC:(oc + 1) * OC],
                                start=(pg == 0), stop=(pg == NKB // 2 - 1),
                                perf_mode=DR)
                # bf16 stream
                for h in range(NH):
                    kb = NH * u + h
                    for mi in range(NMB):
                        lhsT = xT_sb[:, kb * M + mi * MB:
                                     kb * M + (mi + 1) * MB]
                        for oc in range(NOC):
                            sl = slice(h * O_SH + oc * OC,
                                       h * O_SH + (oc + 1) * OC)
                            nc.tensor.matmul(psA[mi][:, oc * OC:(oc + 1) * OC],
                                             lhsT, sgn[:, sl], start=False,
                                             stop=(kb == NKB - 1))

            # DMA order: first weight tiles so dequant starts immediately;
            # xT/x8/bias (needed only by the first matmuls) interleave after.
            dma_w(0)
            nc.scalar.dma_start(xT_sb[:], xT_d[:])
            dma_w(1)
            nc.scalar.dma_start(x8_sb[:], x8_d[:])
            dma_w(2)
            nc.scalar.dma_start(bias_sb[:], bias_d[:])

            # bias opens the psA accumulation group (start=True zeroes psum)
            for mi in range(NMB):
                for oc in range(NOC):
                    sl = slice(oc * OC, (oc + 1) * OC)
                    nc.tensor.matmul(psA[mi][:, sl], ones_sb[:],
                                     bias_sb[:, sl], start=True, stop=False)

            for t in range(NKP + LAG):
                if t < NKP:
                    dma_w(t + PF)
                    wt = wts[t]
                    aq = aqs[t] = aqp.tile([KB, W2], BF, name="aq")
                    sgn = sgns[t] = sp.tile([KB, W2], BF, name="sgn")
                    e = es[t] = ep.tile([KB, W2], BF, name="e")
                    c8 = c8s[t] = cp.tile([KB, NH, O_SH], F8, name="c8")
                    # aq = |q|
                    nc.vector.tensor_scalar(aq[:].bitcast(U16),
                                            wt[:].bitcast(U16),
                                            0x7FFF, None, Alu.bitwise_and)
                    # sgn = q & 0x8000
                    nc.vector.tensor_scalar(sgn[:].bitcast(U16),
                                            wt[:].bitcast(U16),
                                            0x8000, None, Alu.bitwise_and)
                    # E = exp(inv_s*aq + B0)
                    nc.scalar.activation(e[:], aq[:], Act.Exp, bias=b0c[:],
                                         scale=inv_s)
                    # c8 = clamp(q, +-cb) -> fp8e4 on Pool
                    nc.gpsimd.tensor_scalar(c8[:], wt[:], -cb, cb,
                                            Alu.max, Alu.min)
                if t >= LAG:
                    u = t - LAG
                    e = es.pop(u)
                    aqs.pop(u)
                    # E'' = max(E, G) - cb
                    nc.vector.tensor_scalar(e[:], e[:], g, -cb,
                                            Alu.max, Alu.add)
                    # sE = sgn | E''  (u32 bitwise TT; E'' >= 0)
                    nc.vector.tensor_tensor(sgns[u][:].bitcast(U32),
                                            sgns[u][:].bitcast(U32),
                                            e[:].bitcast(U32), Alu.bitwise_or)
                    # pair matmul bursts: issue tiles {u-1, u} together on odd
                    # u so the PE runs longer bursts (p-state ramp amortizes);
                    # keep the final two tiles unpaired to shorten the tail
                    if u % 2 == 1 and u < NKP - 2:
                        do_matmuls(u - 1)
                        do_matmuls(u)
                    elif u >= NKP - 2:
                        do_matmuls(u)

            for mi in range(NMB):
                osb = misc.tile([MB, O_SH], F32, name=f"osb{mi}",
                                tag=f"osb{mi}")
                # out = psA + merge_scale * psB (scale on ACT, add on DVE)
                nc.scalar.mul(osb[:], psB[mi][:], merge_scale)
                nc.vector.tensor_tensor(osb[:], osb[:], psA[mi][:], Alu.add)
                nc.sync.dma_start(out_d[mi * MB:(mi + 1) * MB, :], osb[:])

    nc.compile()
    return nc


def _get_nc(inv_s, b0, g, cb, merge_scale):
    key = (round(inv_s, 12), round(b0, 12), round(g, 12), round(cb, 12),
           round(merge_scale, 12))
    if key not in _CACHE:
        _CACHE[key] = _build(inv_s, b0, g, cb, merge_scale)
    return _CACHE[key]


def _prep_inputs(x, epsilon, gamma, scale, bias, weight_q):
    eps = float(np.asarray(epsilon).ravel()[0])
    gam = float(np.asarray(gamma).ravel()[0])
    sc = float(np.asarray(scale).ravel()[0])
    alpha = (gam - eps) / sc
    assert alpha > 0
    k = eps / alpha
    g = gam / alpha
    cb = 32.0                      # fp8e4m3-exact clamp bound
    sigma = cb / (g - k)           # unit rescale so clamp bound == 32
    inv_s = 1.0 / (sigma * sc)     # = 1/32: exp arg is |q~|/(sigma*sc)
    g_t = g * sigma
    b0 = math.log(g_t) - 1.0
    merge_scale = alpha / sigma

    # x' for the bf16 stream: fold alpha/sigma; x8 raw for the fp8 stream
    xr = np.asarray(x, dtype=np.float32).reshape(M, IN)
    xTb = np.ascontiguousarray(xr.T).reshape(NKB, KB, M).transpose(1, 0, 2)
    xT_blocked = np.ascontiguousarray(xTb * np.float32(alpha / sigma)) \
        .reshape(KB, NKB * M).astype(bfloat16)
    x8_blocked = np.ascontiguousarray(xTb).reshape(KB, NKB * M) \
        .astype(float8_e4m3fn)

    # weights: sigma-scaled bf16, tile-blocked [KB, NKB*O_SH]
    wq = np.asarray(weight_q).astype(np.float32) * np.float32(sigma)
    bias_bf = np.asarray(bias, dtype=np.float32).astype(bfloat16)

    in_maps = []
    for c in range(N_CORES):
        wTc = wq[c * O_SH:(c + 1) * O_SH, :].T          # [IN, O_SH]
        wT_blocked = np.ascontiguousarray(
            wTc.reshape(NKB, KB, O_SH).transpose(1, 0, 2)
        ).reshape(KB, NKB * O_SH).astype(bfloat16)
        in_maps.append({
            "wT": wT_blocked,
            "xT": xT_blocked,
            "x8": x8_blocked,
            "bias": bias_bf[c * O_SH:(c + 1) * O_SH].reshape(1, O_SH),
        })
    return (inv_s, b0, g_t, cb, merge_scale), in_maps


def _run(nc, in_maps, **kw):
    from concourse import bass_utils
    return bass_utils.run_bass_kernel_spmd(
        nc, in_maps, core_ids=list(range(N_CORES)), **kw)


def kernel(x, epsilon, gamma, scale, bias, weight_q):
    consts, in_maps = _prep_inputs(x, epsilon, gamma, scale, bias, weight_q)
    nc = _get_nc(*consts)
    res = _run(nc, in_maps)
    out = np.concatenate(
        [np.asarray(res.results[c]["out"]) for c in range(N_CORES)], axis=1)
    return np.ascontiguousarray(out.reshape(B, S, OUT)).astype(np.float32)


# revision 18
# speedup vs baseline: 1.0491x; 1.0491x over previous
"""Trainium2 Bass kernel for CustomCenterQuantizerLinear.

Computes out = x @ f(weight_q).T + bias over 8 NeuronCores, where f is the
piecewise dequantizer:
    y = q / scale
    f = sign(y) * (eps + |y|*(gam-eps))        for |y| <= 1
    f = sign(y) * gam * exp(|y| - 1)           for |y| >  1
    f = 0                                      for y == 0

Sharding: tensor-parallel column split of weight/bias over out_features
(1024 per core), x replicated.

Math (in f' = f/alpha units, alpha=(gam-eps)/sc, K=eps/alpha, G=gam/alpha):
    f'(q) = s*(max(E,G) - (G-K)) + clamp(q, +-(G-K)),  E = G*exp(|q|/sc - 1)
which is exact for integer q (f'(0)=K, an accepted eps-size error matching
the |y|==0 branch within the rel-err budget).

On-chip, per weight tile (s = sign bit of q):
    sE  = s | (max(exp(|q|*inv_s + B0), G) - (G-K))     [1 exp + 3 TS + 1 TT]
    c8  = fp8e4(clamp(q, +-(G-K)))                      [Pool dual TS]
    out = x' @ sE  (bf16 matmul)  +  x8 @ c8 (fp8 DoubleRow matmul, 2x PE)
with the two PSUM streams merged as psA + (alpha/sigma)*psB.

Weights are pre-scaled on host by sigma = 32/(sc*(1-eps/gam)) so the clamp
bound lands exactly on 32 (representable in fp8e4m3) -- this kills the
systematic fp8 rounding bias at the clamp plateau.
"""

import math
import sys

sys.path.insert(0, "/opt/trn_rl_repo")

import numpy as np
from ml_dtypes import bfloat16, float8_e4m3fn

B, S, IN, OUT = 8, 32, 8192, 8192
N_CORES = 8
M = B * S                 # 256 tokens
O_SH = OUT // N_CORES     # 1024 out features per core
KB = 128                  # contraction block (PE partition dim)
NKB = IN // KB            # 64 k-blocks
MB = 128                  # token block (PSUM partition dim)
NMB = M // MB             # 2 token blocks
OC = 512                  # matmul free-dim chunk (one PSUM bank)
NOC = O_SH // OC          # 2 chunks
NH = 4                    # k-blocks per weight tile
NKP = NKB // NH           # 16 weight tiles
W2 = NH * O_SH            # 4096 free elems per weight tile

_CACHE = {}


def _build(inv_s, b0, g, cb, merge_scale):
    """inv_s/b0/g are in sigma-scaled units; cb is the clamp bound (32)."""
    import concourse.bass as bass
    import concourse.bacc as bacc
    import concourse.mybir as mybir
    import concourse.tile as tile

    BF = mybir.dt.bfloat16
    F32 = mybir.dt.float32
    F8 = mybir.dt.float8e4
    U16 = mybir.dt.uint16
    U32 = mybir.dt.uint32
    Alu = mybir.AluOpType
    Act = mybir.ActivationFunctionType
    DR = mybir.MatmulPerfMode.DoubleRow

    nc = bacc.Bacc("TRN2", target_bir_lowering=False, debug=False,
                   num_devices=N_CORES)
    wT_d = nc.dram_tensor("wT", [KB, NKB * O_SH], BF, kind="ExternalInput").ap()
    xT_d = nc.dram_tensor("xT", [KB, NKB * M], BF, kind="ExternalInput").ap()
    x8_d = nc.dram_tensor("x8", [KB, NKB * M], F8, kind="ExternalInput").ap()
    bias_d = nc.dram_tensor("bias", [1, O_SH], BF, kind="ExternalInput").ap()
    out_d = nc.dram_tensor("out", [M, O_SH], F32, kind="ExternalOutput").ap()

    with tile.TileContext(nc) as tc:
        with (
            tc.tile_pool(name="misc", bufs=1) as misc,
            tc.tile_pool(name="wp", bufs=6) as wp,
            tc.tile_pool(name="aqp", bufs=2) as aqp,
            tc.tile_pool(name="ep", bufs=3) as ep,
            tc.tile_pool(name="sp", bufs=4) as sp,
            tc.tile_pool(name="cp", bufs=4) as cp,
            tc.tile_pool(name="psum", bufs=1, space=bass.MemorySpace.PSUM) as pp,
        ):
            xT_sb = misc.tile([KB, NKB * M], BF)
            x8_sb = misc.tile([KB, NKB, M], F8)
            bias_sb = misc.tile([1, O_SH], BF)
            ones_sb = misc.tile([1, MB], BF)
            b0c = misc.tile([128, 1], F32)
            nc.vector.memset(ones_sb[:], 1.0)
            nc.vector.memset(b0c[:], b0)

            psA = [pp.tile([MB, O_SH], F32, name=f"psA{mi}", tag=f"psA{mi}")
                   for mi in range(NMB)]
            psB = [pp.tile([MB, O_SH], F32, name=f"psB{mi}", tag=f"psB{mi}")
                   for mi in range(NMB)]

            LAG = 1          # stages between prep (aq/sgn/exp/c8) and consume
            PF = 3           # weight-DMA prefetch depth beyond prep stage
            wts, aqs, es, sgns, c8s = {}, {}, {}, {}, {}

            def dma_w(t):
                if t < NKP:
                    wts[t] = wp.tile([KB, W2], BF, name="wt")
                    nc.sync.dma_start(wts[t][:],
                                      wT_d[:, t * W2:(t + 1) * W2])
                    # JIT chunks of xT/x8 for this tile's matmuls (keeps the
                    # big x transfers off the weight-DMA critical path)
                    xs = slice(t * NH * M, (t + 1) * NH * M)
                    nc.scalar.dma_start(xT_sb[:, xs], xT_d[:, xs])
                    nc.scalar.dma_start(x8_sb[:, t * NH:(t + 1) * NH, :],
                                        x8_d[:, xs])

            def do_matmuls(u):
                c8 = c8s.pop(u)
                sgn = sgns.pop(u)
                wts.pop(u)
                # fp8 DoubleRow stream first (inputs ready earliest)
                for i in range(NH // 2):
                    pg = (NH // 2) * u + i
                    for mi in range(NMB):
                        lhs8 = x8_sb[:, 2 * pg:2 * pg + 2,
                                     mi * MB:(mi + 1) * MB]
                        for oc in range(NOC):
                            nc.tensor.matmul(
                                psB[mi][:, oc * OC:(oc + 1) * OC],
                                lhs8,
                                c8[:, 2 * i:2 * i + 2, oc * OC:(oc + 1) * OC],
                                start=(pg == 0), stop=(pg == NKB // 2 - 1),
                                perf_mode=DR)
                # bf16 stream
                for h in range(NH):
                    kb = NH * u + h
                    for mi in range(NMB):
                        lhsT = xT_sb[:, kb * M + mi * MB:
                                     kb * M + (mi + 1) * MB]
                        for oc in range(NOC):
                            sl = slice(h * O_SH + oc * OC,
                                       h * O_SH + (oc + 1) * OC)
                            nc.tensor.matmul(psA[mi][:, oc * OC:(oc + 1) * OC],
                                             lhsT, sgn[:, sl], start=False,
                                             stop=(kb == NKB - 1))

            nc.scalar.dma_start(bias_sb[:], bias_d[:])
            dma_w(0)
            dma_w(1)
            dma_w(2)

            # bias opens the psA accumulation group (start=True zeroes psum)
            for mi in range(NMB):
                for oc in range(NOC):
                    sl = slice(oc * OC, (oc + 1) * OC)
                    nc.tensor.matmul(psA[mi][:, sl], ones_sb[:],
                                     bias_sb[:, sl], start=True, stop=False)

            for t in range(NKP + LAG):
                if t < NKP:
                    dma_w(t + PF)
                    wt = wts[t]
                    aq = aqs[t] = aqp.tile([KB, W2], BF, name="aq")
                    sgn = sgns[t] = sp.tile([KB, W2], BF, name="sgn")
                    e = es[t] = ep.tile([KB, W2], BF, name="e")
                    c8 = c8s[t] = cp.tile([KB, NH, O_SH], F8, name="c8")
                    # aq = |q|
                    nc.vector.tensor_scalar(aq[:].bitcast(U16),
                                            wt[:].bitcast(U16),
                                            0x7FFF, None, Alu.bitwise_and)
                    # sgn = q & 0x8000
                    nc.vector.tensor_scalar(sgn[:].bitcast(U16),
                                            wt[:].bitcast(U16),
                                            0x8000, None, Alu.bitwise_and)
                    # E = exp(inv_s*aq + B0)
                    nc.scalar.activation(e[:], aq[:], Act.Exp, bias=b0c[:],
                                         scale=inv_s)
                    # c8 = clamp(q, +-cb) -> fp8e4 on Pool
                    nc.gpsimd.tensor_scalar(c8[:], wt[:], -cb, cb,
                                            Alu.max, Alu.min)
                if t >= LAG:
                    u = t - LAG
                    e = es.pop(u)
                    aqs.pop(u)
                    # E'' = max(E, G) - cb
                    nc.vector.tensor_scalar(e[:], e[:], g, -cb,
                                            Alu.max, Alu.add)
                    # sE = sgn | E''  (u32 bitwise TT; E'' >= 0)
                    nc.vector.tensor_tensor(sgns[u][:].bitcast(U32),
                                            sgns[u][:].bitcast(U32),
                                            e[:].bitcast(U32), Alu.bitwise_or)
                    # pair matmul bursts: issue tiles {u-1, u} together on odd
                    # u so the PE runs longer bursts (p-state ramp amortizes);
                    # keep the final two tiles unpaired to shorten the tail
                    if u % 2 == 1 and u < NKP - 2:
                        do_matmuls(u - 1)
                        do_matmuls(u)
                    elif u >= NKP - 2:
                        do_matmuls(u)

            for mi in range(NMB):
                osb = misc.tile([MB, O_SH], F32, name=f"osb{mi}",
                                tag=f"osb{mi}")
                # out = psA + merge_scale * psB (scale on ACT, add on DVE)
                nc.scalar.mul(osb[:], psB[mi][:], merge_scale)
                nc.vector.tensor_tensor(osb[:], osb[:], psA[mi][:], Alu.add)
                nc.sync.dma_start(out_d[mi * MB:(mi + 1) * MB, :], osb[:])

    nc.compile()
    return nc


def _get_nc(inv_s, b0, g, cb, merge_scale):
    key = (round(inv_s, 12), round(b0, 12), round(g, 12), round(cb, 12),
           round(merge_scale, 12))
    if key not in _CACHE:
        _CACHE[key] = _build(inv_s, b0, g, cb, merge_scale)
    return _CACHE[key]


def _prep_inputs(x, epsilon, gamma, scale, bias, weight_q):
    eps = float(np.asarray(epsilon).ravel()[0])
    gam = float(np.asarray(gamma).ravel()[0])
    sc = float(np.asarray(scale).ravel()[0])
    alpha = (gam - eps) / sc
    assert alpha > 0
    k = eps / alpha
    g = gam / alpha
    cb = 32.0                      # fp8e4m3-exact clamp bound
    sigma = cb / (g - k)           # unit rescale so clamp bound == 32
    inv_s = 1.0 / (sigma * sc)     # = 1/32: exp arg is |q~|/(sigma*sc)
    g_t = g * sigma
    b0 = math.log(g_t) - 1.0
    merge_scale = alpha / sigma

    # x' for the bf16 stream: fold alpha/sigma; x8 raw for the fp8 stream
    xr = np.asarray(x, dtype=np.float32).reshape(M, IN)
    xTb = np.ascontiguousarray(xr.T).reshape(NKB, KB, M).transpose(1, 0, 2)
    xT_blocked = np.ascontiguousarray(xTb * np.float32(alpha / sigma)) \
        .reshape(KB, NKB * M).astype(bfloat16)
    x8_blocked = np.ascontiguousarray(xTb).reshape(KB, NKB * M) \
        .astype(float8_e4m3fn)

    # weights: sigma-scaled bf16, tile-blocked [KB, NKB*O_SH]
    wq = np.asarray(weight_q).astype(np.float32) * np.float32(sigma)
    bias_bf = np.asarray(bias, dtype=np.float32).astype(bfloat16)

    in_maps = []
    for c in range(N_CORES):
        wTc = wq[c * O_SH:(c + 1) * O_SH, :].T          # [IN, O_SH]
        wT_blocked = np.ascontiguousarray(
            wTc.reshape(NKB, KB, O_SH).transpose(1, 0, 2)
        ).reshape(KB, NKB * O_SH).astype(bfloat16)
        in_maps.append({
            "wT": wT_blocked,
            "xT": xT_blocked,
            "x8": x8_blocked,
            "bias": bias_bf[c * O_SH:(c + 1) * O_SH].reshape(1, O_SH),
        })
    return (inv_s, b0, g_t, cb, merge_scale), in_maps


def _run(nc, in_maps, **kw):
    from concourse import bass_utils
    return bass_utils.run_bass_kernel_spmd(
        nc, in_maps, core_ids=list(range(N_CORES)), **kw)


def kernel(x, epsilon, gamma, scale, bias, weight_q):
    consts, in_maps = _prep_inputs(x, epsilon, gamma, scale, bias, weight_q)
    nc = _get_nc(*consts)
    res = _run(nc, in_maps)
    out = np.concatenate(
        [np.asarray(res.results[c]["out"]) for c in range(N_CORES)], axis=1)
    return np.ascontiguousarray(out.reshape(B, S, OUT)).astype(np.float32)
